# revision 3
# baseline (speedup 1.0000x reference)
"""Trainium2 Bass kernel for nn_LongShortAttention — bf16 rewrite.

Sharding: 8 NeuronCores; core c owns batch c//4, tokens [(c%4)*1024, +1024).
All SBUF data is bf16 (fp32 PSUM accumulation); inputs are converted to
bf16 on the host.  Key differences vs the fp32r baseline:
  - single ACT table set (natural_log_exp_and_others): rstd and 1/Z are
    computed as exp(-0.5*ln(var+eps)) / exp(-ln(Z)); no Sqrt, no table thrash
  - LayerNormed keys also materialized d-major (kvlnT, via PE transpose-back
    of v_ln), so the local sim needs no per-head transposes in phase E
  - sim/exp batched into [128,1024] PSUM tiles; masks applied post-exp with
    batched bf16 tensor_tensor multiplies
  - softmax 1/Z: ln+exp on the Z row, PE ones-matmul broadcast across
    partitions (no DRAM round trip)
  - Q projection emitted after the AllGather so the PE overlaps the collective
"""
import contextlib

import numpy as np
import ml_dtypes

import concourse.bass as bass
import concourse.mybir as mybir
import concourse.tile as tile
from concourse import bacc
import concourse.hw_specs as hw_specs
from concourse.bass_utils import run_bass_kernel_spmd

A = mybir.AluOpType
AF = mybir.ActivationFunctionType
F32 = mybir.dt.float32
BF = mybir.dt.bfloat16

B, N, DIM, H, D = 2, 4096, 1024, 16, 64
W, S, R = 128, 16, 1
EPS = 1e-5
SCALE = D ** -0.5
NC = 8
TOK = 1024
HALO = 128
TOKH = 1152
NT = 9                      # token tiles incl halo (tt=0 is halo)
NSEG = TOKH // S            # 72 segments incl halo
P = 128

# Pin every activation we use (Exp, Ln, Copy, Identity, Square) to the single
# table set that contains them all, so the kernel issues one ACT_TABLE_LOAD
# instead of thrashing between exp/ln/sqrt sets (~1.3us per switch).
_KEEP = {AF.Exp, AF.Ln, AF.Copy, AF.Identity, AF.Square}
_ORIG_TABLES = hw_specs.get_activation_tables


def _patched_tables(arch):
    tabs = _ORIG_TABLES(arch)
    out = {}
    for name, fns in tabs.items():
        if name == "natural_log_exp_and_others":
            out[name] = fns
        else:
            out[name] = fns - _KEEP
    return out


bacc.get_activation_tables = _patched_tables

# expL column layout inside the two [128,1024] psum tiles (tile 0 then 1):
# block u -> (tile, col offset, q start, q len).  Keys of block u are seen by
# queries (u-1)*128 .. (u+1)*128 (first 128 are SELF/causal except u=0).
_LQ = {}
for _u in range(1, 5):
    _LQ[_u] = (0, (_u - 1) * 256, (_u - 1) * 128, 256)
for _u in range(5, 8):
    _LQ[_u] = (1, (_u - 5) * 256, (_u - 1) * 128, 256)
_LQ[8] = (1, 768, 896, 128)
_LQ[0] = (1, 896, 0, 128)


def build_program(nontrivial_ln_l=False, nontrivial_ln_g=False,
                  nonzero_bq=False, nonzero_bkv=False, debug_dump=False):
    nc = bacc.Bacc(None, target_bir_lowering=False, debug=False)

    xt_d = nc.declare_dram_parameter("xt", [DIM, TOKH], BF, isOutput=False)
    wq_d = nc.declare_dram_parameter("wq", [DIM, DIM], BF, isOutput=False)
    wkv_d = nc.declare_dram_parameter("wkv", [DIM, DIM], BF, isOutput=False)
    wo_d = nc.declare_dram_parameter("wo", [DIM, DIM], BF, isOutput=False)
    ident_d = nc.declare_dram_parameter("ident", [P, P], BF, isOutput=False)
    seg16_d = nc.declare_dram_parameter("seg16", [P, 8], BF, isOutput=False)
    tri8_d = nc.declare_dram_parameter("tri8", [P, 8, P], BF, isOutput=False)
    halo_d = nc.declare_dram_parameter("halom", [P, P], BF, isOutput=False)
    gmask_d = nc.declare_dram_parameter("gmask", [P, 2, 2, 512], BF,
                                        isOutput=False)
    stats_d = nc.declare_dram_parameter("stats_lhsT", [P, 4], BF,
                                        isOutput=False)
    stats2_d = nc.declare_dram_parameter("stats2_lhsT", [P, 2], BF,
                                         isOutput=False)
    ones1_d = nc.declare_dram_parameter("ones1", [1, D], BF, isOutput=False)
    if nonzero_bq:
        bq_d = nc.declare_dram_parameter("bqs", [P, 8], F32, isOutput=False)
    if nonzero_bkv:
        bkv_d = nc.declare_dram_parameter("bkvs", [P, 8], F32, isOutput=False)
    if nontrivial_ln_l:
        lnl_row_d = nc.declare_dram_parameter("lnl_row", [P, 2, D], BF,
                                              isOutput=False)
    if nontrivial_ln_g:
        lng_row_d = nc.declare_dram_parameter("lng_row", [64, 2, D], F32,
                                              isOutput=False)
    out_d = nc.declare_dram_parameter("out", [8, P, DIM], F32, isOutput=True)
    if debug_dump:
        dbg = {}
        for nm, shp in (("dbg_kvT", [P, 8, TOKH]), ("dbg_kvlnT", [P, 8, TOKH]),
                        ("dbg_qT", [P, 8, TOK]), ("dbg_vln", [P, NT, 16, 65]),
                        ("dbg_gv", [P, 2, 16, 65]), ("dbg_gkvT", [P, 2, 8, P]),
                        ("dbg_attnT", [P, 8, TOK]),
                        ("dbg_glnin", [64, 16, 64]), ("dbg_glnout", [64, 16, 64]),
                        ("dbg_gst", [8, 8, 8, P]), ("dbg_gkvr", [64, 16, 64]),
                        ("dbg_rcol", [P, NT, 16]), ("dbg_bcol", [P, NT, 16]),
                        ("dbg_pcol", [P, NT, 16])):
            dbg[nm] = nc.declare_dram_parameter(nm, shp, BF if nm not in
                                                ("dbg_rcol", "dbg_bcol",
                                                 "dbg_pcol") else F32,
                                                isOutput=True)

    with tile.TileContext(nc) as tc:
        stack = contextlib.ExitStack()
        with stack:
            dram = stack.enter_context(
                tc.tile_pool(name="dram", bufs=1, space="DRAM"))
            consts = stack.enter_context(tc.tile_pool(name="consts", bufs=1))

            perm = stack.enter_context(tc.tile_pool(name="perm", bufs=1))

            ident = consts.tile([P, P], BF)
            seg16 = consts.tile([P, 8], BF)
            tri8 = consts.tile([P, 8, P], BF)
            halom = consts.tile([P, P], BF)
            gmask = consts.tile([P, 2, 2, 512], BF)
            stats_lhsT = consts.tile([P, 4], BF)
            nc.sync.dma_start(out=stats_lhsT[:], in_=stats_d[:])
            stats2_lhsT = consts.tile([P, 2], BF)
            nc.sync.dma_start(out=stats2_lhsT[:], in_=stats2_d[:])
            ones1 = consts.tile([1, D], BF)
            nc.sync.dma_start(out=ones1[:], in_=ones1_d[:])
            eps72 = consts.tile([NSEG, 1], F32)
            nc.vector.memset(eps72[:], EPS)
            eps64 = consts.tile([64, 1], F32)
            nc.vector.memset(eps64[:], EPS)
            if nonzero_bq:
                bqs = consts.tile([P, 8], F32)
                nc.sync.dma_start(out=bqs[:], in_=bq_d[:])
            if nonzero_bkv:
                bkvs = consts.tile([P, 8], F32)
                nc.sync.dma_start(out=bkvs[:], in_=bkv_d[:])
            if nontrivial_ln_l:
                lnl_row = consts.tile([P, 2, D], BF)
                nc.sync.dma_start(out=lnl_row[:], in_=lnl_row_d[:])
            if nontrivial_ln_g:
                lng_row = consts.tile([64, 2, D], F32)
                nc.sync.dma_start(out=lng_row[:], in_=lng_row_d[:])

            qT = perm.tile([P, 8, TOK], BF)           # [dim-in-m, m, tok]
            kvT = perm.tile([P, 8, TOKH], BF)         # raw kv, d-major
            kvlnT = perm.tile([P, 8, TOKH], BF)       # LayerNormed, d-major
            attnT = perm.tile([P, 8, TOK], BF)
            v_ln = perm.tile([P, NT, 16, 65], BF)
            gv = perm.tile([P, 2, 16, 65], BF)
            gkvT = perm.tile([P, 2, 8, P], BF)

            # ---------------- Phase B: KV projection + stats ----------------
            sdram_kv = dram.tile([4, 8, TOKH], F32)   # mu_h0, z_h0, mu_h1, z_h1
            sdram_sq = dram.tile([2, 8, TOKH], F32)   # e2_h0, e2_h1
            bq_stack = contextlib.ExitStack()
            xw_pool = bq_stack.enter_context(tc.tile_pool(name="xw", bufs=8))
            wld_pool = bq_stack.enter_context(tc.tile_pool(name="wld", bufs=8))

            xt_k = []
            for k in range(8):
                xk = xw_pool.tile([P, TOKH], BF, tag="xk")
                nc.sync.dma_start(out=xk[:], in_=xt_d[k * P:(k + 1) * P, :])
                xt_k.append(xk)

            with tc.tile_pool(name="sq", bufs=2) as sq_pool, \
                 tc.tile_pool(name="zstage", bufs=2) as zstage_pool, \
                 tc.tile_pool(name="pproj", bufs=2, space="PSUM") as pproj, \
                 tc.tile_pool(name="pz", bufs=1, space="PSUM") as pz:
                wkv_k = []
                for k in range(8):
                    wk2 = wld_pool.tile([P, DIM], BF, tag="wmat")
                    nc.sync.dma_start(out=wk2[:],
                                      in_=wkv_d[k * P:(k + 1) * P, :])
                    wkv_k.append(wk2)

                for m in range(8):
                    for nt3 in range(3):
                        ps = pproj.tile([P, 512], F32, tag="proj")
                        for k in range(8):
                            nc.tensor.matmul(
                                ps[:, :384],
                                wkv_k[k][:, m * P:(m + 1) * P],
                                xt_k[k][:, nt3 * 384:nt3 * 384 + 384],
                                start=(k == 0), stop=(k == 7))
                        dst = kvT[:, m, nt3 * 384:(nt3 + 1) * 384]
                        if nonzero_bkv:
                            nc.scalar.activation(dst, ps[:, :384], AF.Identity,
                                                 bias=bkvs[:, m:m + 1])
                        else:
                            nc.scalar.activation(dst, ps[:, :384], AF.Copy)
                    # stats matmuls for this m
                    sqt = sq_pool.tile([P, TOKH], BF, tag="sqt")
                    with nc.allow_low_precision(reason="bf16 square"):
                        nc.vector.tensor_tensor(out=sqt[:], in0=kvT[:, m, :],
                                                in1=kvT[:, m, :], op=A.mult)
                    psz = pz.tile([4, 3, 512], F32, tag="zp")
                    psz2 = pz.tile([2, 3, 512], F32, tag="zp2")
                    for nt3 in range(3):
                        nc.tensor.matmul(
                            psz[:, nt3, :384], stats_lhsT[:],
                            kvT[:, m, nt3 * 384:nt3 * 384 + 384],
                            start=True, stop=True)
                        nc.tensor.matmul(
                            psz2[:, nt3, :384], stats2_lhsT[:],
                            sqt[:, nt3 * 384:nt3 * 384 + 384],
                            start=True, stop=True)
                    zst = zstage_pool.tile([4, TOKH], F32, tag="zst")
                    nc.scalar.activation(
                        zst[:].rearrange("p (a b) -> p a b", a=3, b=384),
                        psz[:, :, :384], AF.Copy)
                    nc.sync.dma_start(out=sdram_kv[:, m, :], in_=zst[:])
                    zst2 = zstage_pool.tile([2, TOKH], F32, tag="zst2")
                    nc.scalar.activation(
                        zst2[:].rearrange("p (a b) -> p a b", a=3, b=384),
                        psz2[:, :, :384], AF.Copy)
                    nc.sync.dma_start(out=sdram_sq[:, m, :], in_=zst2[:])

                wq_k = []
                for k in range(8):
                    wk3 = wld_pool.tile([P, DIM], BF, tag="wmat")
                    nc.sync.dma_start(out=wk3[:],
                                      in_=wq_d[k * P:(k + 1) * P, :])
                    wq_k.append(wk3)
                for m in range(4):
                    for nt2 in range(2):
                        ps = pproj.tile([P, 512], F32, tag="proj")
                        for k in range(8):
                            nc.tensor.matmul(
                                ps[:],
                                wq_k[k][:, m * P:(m + 1) * P],
                                xt_k[k][:, HALO + nt2 * 512:
                                        HALO + nt2 * 512 + 512],
                                start=(k == 0), stop=(k == 7))
                        dst = qT[:, m, nt2 * 512:(nt2 + 1) * 512]
                        if nonzero_bq:
                            nc.scalar.activation(dst, ps[:], AF.Identity,
                                                 bias=bqs[:, m:m + 1])
                        else:
                            nc.scalar.activation(dst, ps[:], AF.Copy)

            nc.sync.dma_start(out=ident[:], in_=ident_d[:])
            nc.sync.dma_start(out=seg16[:], in_=seg16_d[:])
            nc.sync.dma_start(out=tri8[:], in_=tri8_d[:])
            nc.sync.dma_start(out=halom[:], in_=halo_d[:])
            nc.sync.dma_start(out=gmask[:], in_=gmask_d[:])

            # ---------------- Phase C: z softmax + rstd/bcol ----------------
            pcol = perm.tile([P, NT, 16], F32)
            rstd_col = perm.tile([P, NT, 16], F32)
            bcol_col = perm.tile([P, NT, 16], F32)
            with tc.tile_pool(name="zseg", bufs=1) as zseg_pool:
                zseg = zseg_pool.tile([NSEG, 16, S], F32)
                mseg = zseg_pool.tile([NSEG, 16, S], F32)
                eseg = zseg_pool.tile([NSEG, 16, S], F32)
                for par in range(2):
                    nc.gpsimd.dma_start(
                        out=zseg[:, par::2, :],
                        in_=sdram_kv[2 * par + 1].rearrange(
                            "m (g s) -> g m s", s=S))
                    nc.gpsimd.dma_start(
                        out=mseg[:, par::2, :],
                        in_=sdram_kv[2 * par].rearrange(
                            "m (g s) -> g m s", s=S))
                    nc.gpsimd.dma_start(
                        out=eseg[:, par::2, :],
                        in_=sdram_sq[par].rearrange(
                            "m (g s) -> g m s", s=S))
                # segment softmax of z first (pcol gates the compress branch)
                ez = zseg_pool.tile([NSEG, 16, S], F32)
                nc.scalar.activation(ez[:], zseg[:], AF.Exp)
                sz = zseg_pool.tile([NSEG, 16], F32)
                nc.vector.reduce_sum(sz[:], ez[:], axis=mybir.AxisListType.X)
                lsz = zseg_pool.tile([NSEG, 16], F32)
                nc.scalar.activation(lsz[:], sz[:], AF.Ln)
                rz = zseg_pool.tile([NSEG, 16], F32)
                nc.scalar.activation(rz[:], lsz[:], AF.Exp, scale=-1.0)
                pseg_sh = zseg_pool.tile([NSEG, S, 16], F32)
                for h in range(16):
                    nc.vector.tensor_scalar_mul(
                        pseg_sh[:, :, h], ez[:, h, :], rz[:, h:h + 1])
                pseg_dram = dram.tile([NSEG, S, 16], F32)
                nc.sync.dma_start(out=pseg_dram[:], in_=pseg_sh[:])
                nc.gpsimd.dma_start(
                    out=pcol[:],
                    in_=pseg_dram[:].rearrange("(t g) s h -> (g s) t h", g=8))
                # var = E[x^2] - mu^2 ; rstd = exp(-0.5*ln(var+eps))
                var = zseg_pool.tile([NSEG, 16, S], F32)
                nc.vector.scalar_tensor_tensor(
                    out=var[:], in0=mseg[:], scalar=1.0, in1=mseg[:],
                    op0=A.mult, op1=A.mult)
                nc.vector.tensor_tensor(out=var[:], in0=eseg[:], in1=var[:],
                                        op=A.subtract)
                lnv = zseg_pool.tile([NSEG, 16, S], F32)
                nc.scalar.activation(lnv[:], var[:], AF.Ln, bias=eps72[:])
                rs = zseg_pool.tile([NSEG, 16, S], F32)
                nc.scalar.activation(rs[:], lnv[:], AF.Exp, scale=-0.5)
                rs_sh = zseg_pool.tile([NSEG, S, 16], F32)
                bc_sh = zseg_pool.tile([NSEG, S, 16], F32)
                for h in range(16):
                    nc.vector.tensor_copy(rs_sh[:, :, h], rs[:, h, :])
                    nc.vector.scalar_tensor_tensor(
                        out=bc_sh[:, :, h], in0=mseg[:, h, :], scalar=-1.0,
                        in1=rs[:, h, :], op0=A.mult, op1=A.mult)
                rs_dram = dram.tile([NSEG, S, 16], F32)
                bc_dram = dram.tile([NSEG, S, 16], F32)
                nc.sync.dma_start(out=rs_dram[:], in_=rs_sh[:])
                nc.sync.dma_start(out=bc_dram[:], in_=bc_sh[:])
                for src, dst in ((rs_dram, rstd_col), (bc_dram, bcol_col)):
                    nc.gpsimd.dma_start(
                        out=dst[:],
                        in_=src[:].rearrange("(t g) s h -> (g s) t h", g=8))

            # ---------- Phase D: transposes, v_ln, kvlnT, compress ----------
            nc.vector.memset(v_ln[:, :, :, 64], 1.0)
            nc.vector.memset(gv[:, :, :, 64], 1.0)

            gkvr_dram = dram.tile([8, 8, 8, P], BF)   # [m, g, tt-1, (par d)]
            with tc.tile_pool(name="ptok", bufs=4, space="PSUM") as ptokp, \
                 tc.tile_pool(name="pg", bufs=2, space="PSUM") as pgp, \
                 tc.tile_pool(name="gst", bufs=2) as gst_pool, \
                 tc.tile_pool(name="wscr", bufs=4) as wscrp:
                for m in range(8):
                    pgm = pgp.tile([8, 8, P], F32, tag="pgm")
                    for tt in range(1, NT):
                        ptok = ptokp.tile([P, P], BF, tag="ptok")
                        nc.tensor.transpose(
                            ptok[:], kvT[:, m, tt * P:(tt + 1) * P], ident[:])
                        wscr = wscrp.tile([P, P], BF, tag="wscr")
                        for par in range(2):
                            h = 2 * m + par
                            hs = ptok[:, par * 64:(par + 1) * 64]
                            with nc.allow_low_precision(reason="bf16 store"):
                                nc.vector.tensor_scalar_mul(
                                    wscr[:, par * 64:(par + 1) * 64], hs,
                                    pcol[:, tt, h:h + 1])
                        nc.tensor.matmul(pgm[:, tt - 1, :], seg16[:],
                                         wscr[:], start=True, stop=True)
                    gst = gst_pool.tile([8, 8, P], BF, tag="gst")
                    with nc.allow_low_precision(reason="bf16 store"):
                        nc.scalar.activation(gst[:], pgm[:], AF.Copy)
                    nc.sync.dma_start(out=gkvr_dram[m], in_=gst[:])
                    if debug_dump:
                        nc.sync.dma_start(out=dbg["dbg_gst"][m], in_=gst[:])

            # ---- own-seg LN of compressed kv, AllGather across batch group --
            with tc.tile_pool(name="gln", bufs=1) as gln_pool:
                # glnin partition p = local segment in (g, t) order:
                # actual segment index = (p % 8) * 8 + p // 8
                glnin = gln_pool.tile([64, 16, 64], BF)
                for m in range(8):
                    nc.gpsimd.dma_start(
                        out=glnin[:, 2 * m:2 * m + 2, :],
                        in_=gkvr_dram[m].rearrange("g t (p d) -> (g t) p d",
                                                   p=2))
                glnout = gln_pool.tile([64, 16, 64], BF)
                st2 = gln_pool.tile([64, 16, 6], F32)
                for h in range(16):
                    nc.vector.bn_stats(out=st2[:, h, :], in_=glnin[:, h, :])
                mv2 = gln_pool.tile([64, 16, 2], F32)
                for h in range(16):
                    nc.vector.bn_aggr(out=mv2[:, h, :], in_=st2[:, h, :])
                lnv2 = gln_pool.tile([64, 16], F32)
                nc.scalar.activation(lnv2[:], mv2[:, :, 1], AF.Ln,
                                     bias=eps64[:])
                rstd2 = gln_pool.tile([64, 16], F32)
                nc.scalar.activation(rstd2[:], lnv2[:], AF.Exp, scale=-0.5)
                bcol2 = gln_pool.tile([64, 16], F32)
                nc.vector.scalar_tensor_tensor(
                    out=bcol2[:], in0=mv2[:, :, 0], scalar=-1.0, in1=rstd2[:],
                    op0=A.mult, op1=A.mult)
                for h in range(16):
                    with nc.allow_low_precision(reason="bf16 store"):
                        nc.vector.tensor_scalar(
                            out=glnout[:, h, :], in0=glnin[:, h, :],
                            scalar1=rstd2[:, h:h + 1],
                            scalar2=bcol2[:, h:h + 1],
                            op0=A.mult, op1=A.add)
                        if nontrivial_ln_g:
                            nc.vector.tensor_tensor(
                                out=glnout[:, h, :], in0=glnout[:, h, :],
                                in1=lng_row[:, 0, :], op=A.mult)
                            nc.vector.tensor_tensor(
                                out=glnout[:, h, :], in0=glnout[:, h, :],
                                in1=lng_row[:, 1, :], op=A.add)

                if debug_dump:
                    nc.sync.dma_start(out=dbg["dbg_gkvr"][:], in_=glnin[:])
                    glnin_dbg = perm.tile([64, 16, 64], BF)
                    nc.vector.tensor_copy(glnin_dbg[:], glnin[:])
                    glnout_dbg = perm.tile([64, 16, 64], BF)
                    nc.vector.tensor_copy(glnout_dbg[:], glnout[:])
                cc_in = dram.tile([16, 64, 64], BF)
                nc.sync.dma_start(out=cc_in[:].transpose([1, 0, 2]),
                                  in_=glnout[:])
                cc_out = dram.tile([4, 16, 64, 64], BF)
                nc.gpsimd.collective_compute(
                    "AllGather", A.bypass,
                    replica_groups=[[0, 1, 2, 3], [4, 5, 6, 7]],
                    ins=[cc_in.opt()], outs=[cc_out.opt()])
                for bb in range(2):
                    for cg in range(2):
                        nc.sync.dma_start(
                            out=gv[64 * cg:64 * cg + 64, bb, :, 0:64],
                            in_=cc_out[2 * bb + cg].transpose([1, 0, 2]))

            # ---- D2: v_ln + kvlnT (overlaps the AllGather) ----
            with tc.tile_pool(name="ptok2", bufs=4, space="PSUM") as ptokp2, \
                 tc.tile_pool(name="vtmps", bufs=4) as vtmpp:
                for m in range(8):
                    for tt in range(NT):
                        ptok = ptokp2.tile([P, P], BF, tag="ptok")
                        nc.tensor.transpose(
                            ptok[:], kvT[:, m, tt * P:(tt + 1) * P], ident[:])
                        vtmp = vtmpp.tile([P, P], BF, tag="vtmp")
                        for par in range(2):
                            h = 2 * m + par
                            hs = ptok[:, par * 64:(par + 1) * 64]
                            with nc.allow_low_precision(reason="bf16 store"):
                                nc.vector.tensor_scalar(
                                    out=vtmp[:, par * 64:(par + 1) * 64],
                                    in0=hs,
                                    scalar1=rstd_col[:, tt, h:h + 1],
                                    scalar2=bcol_col[:, tt, h:h + 1],
                                    op0=A.mult, op1=A.add)
                                if nontrivial_ln_l:
                                    nc.vector.tensor_tensor(
                                        out=vtmp[:, par * 64:(par + 1) * 64],
                                        in0=vtmp[:, par * 64:(par + 1) * 64],
                                        in1=lnl_row[:, 0, :], op=A.mult)
                                    nc.vector.tensor_tensor(
                                        out=vtmp[:, par * 64:(par + 1) * 64],
                                        in0=vtmp[:, par * 64:(par + 1) * 64],
                                        in1=lnl_row[:, 1, :], op=A.add)
                        nc.vector.tensor_copy(
                            v_ln[:, tt, 2 * m:2 * m + 2, 0:64],
                            vtmp[:].rearrange("p (a b) -> p a b", a=2, b=64))
                        ptok2 = ptokp2.tile([P, P], BF, tag="ptok")
                        nc.tensor.transpose(ptok2[:], vtmp[:], ident[:])
                        nc.vector.tensor_copy(
                            kvlnT[:, m, tt * P:(tt + 1) * P], ptok2[:])

            # ------- Q projection (emitted here to overlap the AllGather) ----
            with tc.tile_pool(name="pproj2", bufs=3, space="PSUM") as pproj2:
                for m in range(4, 6):
                    for nt2 in range(2):
                        ps = pproj2.tile([P, 512], F32, tag="proj2")
                        for k in range(8):
                            nc.tensor.matmul(
                                ps[:],
                                wq_k[k][:, m * P:(m + 1) * P],
                                xt_k[k][:, HALO + nt2 * 512:
                                        HALO + nt2 * 512 + 512],
                                start=(k == 0), stop=(k == 7))
                        dst = qT[:, m, nt2 * 512:(nt2 + 1) * 512]
                        if nonzero_bq:
                            nc.scalar.activation(dst, ps[:], AF.Identity,
                                                 bias=bqs[:, m:m + 1])
                        else:
                            nc.scalar.activation(dst, ps[:], AF.Copy)

            with tc.tile_pool(name="pgt", bufs=2, space="PSUM") as pgt:
                for bb in range(2):
                    for mg in range(2):
                        pst = pgt.tile([64, 4, P], BF, tag="pgt")
                        pst2 = pgt.tile([64, 4, P], BF, tag="pgt2")
                        for j in range(4):
                            m = 4 * mg + j
                            nc.tensor.transpose(pst[:, j, :],
                                                gv[:, bb, 2 * m, 0:64],
                                                ident[:])
                            nc.tensor.transpose(pst2[:, j, :],
                                                gv[:, bb, 2 * m + 1, 0:64],
                                                ident[:])
                        nc.vector.tensor_copy(
                            gkvT[0:64, bb, 4 * mg:4 * mg + 4, :], pst[:])
                        nc.vector.tensor_copy(
                            gkvT[64:128, bb, 4 * mg:4 * mg + 4, :], pst2[:])

            with tc.tile_pool(name="pproj3", bufs=3, space="PSUM") as pproj3:
                for m in range(6, 8):
                    for nt2 in range(2):
                        ps = pproj3.tile([P, 512], F32, tag="proj3")
                        for k in range(8):
                            nc.tensor.matmul(
                                ps[:],
                                wq_k[k][:, m * P:(m + 1) * P],
                                xt_k[k][:, HALO + nt2 * 512:
                                        HALO + nt2 * 512 + 512],
                                start=(k == 0), stop=(k == 7))
                        dst = qT[:, m, nt2 * 512:(nt2 + 1) * 512]
                        if nonzero_bq:
                            nc.scalar.activation(dst, ps[:], AF.Identity,
                                                 bias=bqs[:, m:m + 1])
                        else:
                            nc.scalar.activation(dst, ps[:], AF.Copy)

            bq_stack.close()

            wo_k = []
            for k in range(8):
                wk4 = perm.tile([P, DIM], BF, tag=f"wo{k}")
                nc.sync.dma_start(out=wk4[:], in_=wo_d[k * P:(k + 1) * P, :])
                wo_k.append(wk4)

            # ---------------- Phase E: attention per head-pair ---------------
            zr_dram = dram.tile([16, TOK], BF)
            with tc.tile_pool(name="expl", bufs=3) as explp, \
                 tc.tile_pool(name="expg", bufs=3) as expgp, \
                 tc.tile_pool(name="zrp", bufs=2) as zrp, \
                 tc.tile_pool(name="psim", bufs=2, space="PSUM") as psim, \
                 tc.tile_pool(name="pav", bufs=2, space="PSUM") as pav:
                for m in range(8):
                    for par in range(2):
                        h = 2 * m + par
                        prow = slice(par * 64, par * 64 + 64)
                        # ---- local sim ----
                        plsA = psim.tile([P, 1024], F32, tag="sim")
                        plsB = psim.tile([P, 1024], F32, tag="sim")
                        for u in range(NT):
                            t_i, co, qs, qn = _LQ[u]
                            pls = plsA if t_i == 0 else plsB
                            nc.tensor.matmul(
                                pls[:, co:co + qn],
                                kvlnT[prow, m, u * P:(u + 1) * P],
                                qT[prow, m, qs:qs + qn],
                                start=True, stop=True)
                        expL = explp.tile([P, 2048], BF, tag="expL")
                        with nc.allow_low_precision(reason="bf16 store"):
                            nc.scalar.activation(expL[:, 0:1024], plsA[:],
                                                 AF.Exp)
                            nc.scalar.activation(expL[:, 1024:2048], plsB[:],
                                                 AF.Exp)
                        # masks: SELF cols of u1..u4, u5..u7, u8; halo u0
                        eL4 = expL[:, 0:1024].rearrange("p (a b) -> p a b",
                                                        a=4, b=256)
                        nc.vector.tensor_tensor(
                            out=eL4[:, :, 0:128], in0=eL4[:, :, 0:128],
                            in1=tri8[:, 0:4, :], op=A.mult)
                        eL3 = expL[:, 1024:2048].rearrange("p (a b) -> p a b",
                                                           a=4, b=256)
                        nc.vector.tensor_tensor(
                            out=eL3[:, :, 0:128], in0=eL3[:, :, 0:128],
                            in1=tri8[:, 4:8, :], op=A.mult)
                        nc.vector.tensor_tensor(
                            out=expL[:, 1920:2048], in0=expL[:, 1920:2048],
                            in1=halom[:], op=A.mult)
                        # ---- global sim ----
                        expG = expgp.tile([P, 2, 2, 512], BF, tag="expG")
                        for Q in range(2):
                            pgs = psim.tile([P, 1024], F32, tag="sim")
                            for bb in range(2):
                                nc.tensor.matmul(
                                    pgs[:, bb * 512:(bb + 1) * 512],
                                    gkvT[prow, bb, m, :],
                                    qT[prow, m, Q * 512:(Q + 1) * 512],
                                    start=True, stop=True)
                            with nc.allow_low_precision(reason="bf16 store"):
                                nc.scalar.activation(expG[:, Q, :, :], pgs[:],
                                                     AF.Exp)
                        nc.vector.tensor_tensor(
                            out=expG[:], in0=expG[:], in1=gmask[:],
                            op=A.mult)
                        # ---- AV + Z ----
                        avp = pav.tile([65, 2, 512], F32, tag="avp")
                        for Q in range(2):
                            first = True
                            for bb in range(2):
                                nc.tensor.matmul(
                                    avp[:, Q, :], gv[:, bb, h, :],
                                    expG[:, Q, bb, :],
                                    start=first, stop=False)
                                first = False
                            for u in range(4 * Q, 4 * Q + 5):
                                t_i, co, qs, qn = _LQ[u]
                                lo = max(qs, Q * 512)
                                hi = min(qs + qn, Q * 512 + 512)
                                src = expL[:, t_i * 1024 + co + (lo - qs):
                                           t_i * 1024 + co + (hi - qs)]
                                nc.tensor.matmul(
                                    avp[:, Q, lo - Q * 512:hi - Q * 512],
                                    v_ln[:, u, h, :], src,
                                    start=False, stop=(u == 4 * Q + 4))
                        # 1/Z = exp(-ln(Z)), broadcast via PE ones-matmul
                        zl = zrp.tile([1, TOK], F32, tag="zl")
                        nc.scalar.activation(zl[:], avp[64:65, :, :], AF.Ln)
                        zr = zrp.tile([1, TOK], BF, tag="zr")
                        with nc.allow_low_precision(reason="bf16 store"):
                            nc.scalar.activation(zr[:], zl[:], AF.Exp,
                                                 scale=-1.0)
                        nc.sync.dma_start(out=zr_dram[h, :], in_=zr[:])
                        zrb = zrp.tile([64, TOK], BF, tag="zrb")
                        nc.gpsimd.dma_start(
                            out=zrb[:],
                            in_=zr_dram[h, :].unsqueeze(0)
                            .partition_broadcast(64))
                        with nc.allow_low_precision(reason="bf16 store"):
                            nc.vector.tensor_tensor(
                                out=attnT[prow, m, :],
                                in0=avp[0:64, :, :].rearrange(
                                    "p a b -> p (a b)"),
                                in1=zrb[:], op=A.mult)

            if debug_dump:
                nc.sync.dma_start(out=dbg["dbg_kvT"][:], in_=kvT[:])
                nc.sync.dma_start(out=dbg["dbg_kvlnT"][:], in_=kvlnT[:])
                nc.sync.dma_start(out=dbg["dbg_qT"][:], in_=qT[:])
                nc.sync.dma_start(out=dbg["dbg_vln"][:], in_=v_ln[:])
                nc.sync.dma_start(out=dbg["dbg_gv"][:], in_=gv[:])
                nc.sync.dma_start(out=dbg["dbg_gkvT"][:], in_=gkvT[:])
                nc.sync.dma_start(out=dbg["dbg_attnT"][:], in_=attnT[:])
                nc.sync.dma_start(out=dbg["dbg_glnin"][:], in_=glnin_dbg[:])
                nc.sync.dma_start(out=dbg["dbg_glnout"][:], in_=glnout_dbg[:])
                nc.sync.dma_start(out=dbg["dbg_rcol"][:], in_=rstd_col[:])
                nc.sync.dma_start(out=dbg["dbg_bcol"][:], in_=bcol_col[:])
                nc.sync.dma_start(out=dbg["dbg_pcol"][:], in_=pcol[:])

            # ---------------- Phase F: final projection ----------------
            with tc.tile_pool(name="pf", bufs=2, space="PSUM") as pf, \
                 tc.tile_pool(name="outp", bufs=2) as outp:
                for tt in range(8):
                    psf = pf.tile([P, 2, 512], F32, tag="psf")
                    for nh in range(2):
                        for m in range(8):
                            nc.tensor.matmul(
                                psf[:, nh, :],
                                attnT[:, m, tt * P:(tt + 1) * P],
                                wo_k[m][:, nh * 512:(nh + 1) * 512],
                                start=(m == 0), stop=(m == 7))
                    ot = outp.tile([P, DIM], F32, tag="ot")
                    nc.scalar.activation(
                        ot[:].rearrange("p (a b) -> p a b", a=2, b=512),
                        psf[:], AF.Copy)
                    nc.sync.dma_start(out=out_d[tt], in_=ot[:])

    nc.compile()
    return nc


_PROG_CACHE = {}


def _get_program(key):
    if key not in _PROG_CACHE:
        _PROG_CACHE[key] = build_program(*key)
    return _PROG_CACHE[key]


def _bf16(x):
    return np.ascontiguousarray(np.asarray(x, dtype=np.float32).astype(
        ml_dtypes.bfloat16))


def _f32(x):
    return np.ascontiguousarray(x, dtype=np.float32)


def _host_constants(Wp):
    ident = np.eye(P, dtype=np.float32)
    seg16 = np.zeros((P, 8), np.float32)
    for g in range(8):
        seg16[g * 16:(g + 1) * 16, g] = 1.0
    jk, ii = np.meshgrid(np.arange(P), np.arange(P), indexing="ij")
    tri = (jk <= ii).astype(np.float32)
    tri8 = np.ascontiguousarray(np.broadcast_to(tri[:, None, :], (P, 8, P)))
    stats_lhsT = np.zeros((P, 4), np.float32)
    stats_lhsT[0:64, 0] = 1.0 / 64
    stats_lhsT[0:64, 1] = Wp[:, 0]
    stats_lhsT[64:128, 2] = 1.0 / 64
    stats_lhsT[64:128, 3] = Wp[:, 0]
    stats2_lhsT = np.zeros((P, 2), np.float32)
    stats2_lhsT[0:64, 0] = 1.0 / 64
    stats2_lhsT[64:128, 1] = 1.0 / 64
    ones1 = np.ones((1, D), np.float32)
    return ident, seg16, tri8, stats_lhsT, stats2_lhsT, ones1


def kernel(x, Wq, bq, Wkv, bkv, Wp, bp, ln_l_g, ln_l_b, ln_g_g, ln_g_b, Wo, bo):
    # NOTE: bp shifts all segment logits equally (R=1), so the segment softmax
    # is invariant to it; it is deliberately unused.  bo is added on the host.
    x = _f32(x); Wq = _f32(Wq); Wkv = _f32(Wkv); Wo = _f32(Wo)
    bq = _f32(bq); bkv = _f32(bkv); bo = _f32(bo); Wp = _f32(Wp)
    ln_l_g = _f32(ln_l_g); ln_l_b = _f32(ln_l_b)
    ln_g_g = _f32(ln_g_g); ln_g_b = _f32(ln_g_b)

    nontrivial_ln_l = not (np.all(ln_l_g == 1.0) and np.all(ln_l_b == 0.0))
    nontrivial_ln_g = not (np.all(ln_g_g == 1.0) and np.all(ln_g_b == 0.0))
    nonzero_bq = bool(np.any(bq != 0.0))
    nonzero_bkv = bool(np.any(bkv != 0.0))
    key = (nontrivial_ln_l, nontrivial_ln_g, nonzero_bq, nonzero_bkv)
    nc = _get_program(key)

    (ident, seg16, tri8, stats_lhsT, stats2_lhsT,
     ones1) = _host_constants(Wp)

    wq_b = _bf16(Wq * SCALE)
    wkv_b = _bf16(Wkv)
    wo_b = _bf16(Wo)
    ident_b = _bf16(ident); seg16_b = _bf16(seg16); tri8_b = _bf16(tri8)
    stats_b = _bf16(stats_lhsT); stats2_b = _bf16(stats2_lhsT)
    ones1_b = _bf16(ones1)

    in_maps = []
    for c in range(NC):
        bc, ci = c // 4, c % 4
        tc0 = ci * TOK
        xb = x[bc]
        xtc = np.zeros((DIM, TOKH), np.float32)
        lo = tc0 - HALO
        src_lo = max(lo, 0)
        xtc[:, src_lo - lo:] = xb[src_lo:tc0 + TOK].T
        halom = (np.ones if ci > 0 else np.zeros)((P, P)).astype(np.float32)
        # gmask [seg128, Q2, bb2, tok512]: allowed = qabs >= 16*segabs+15
        qi = tc0 + np.arange(TOK).reshape(2, 512)
        # segment order within each rank block of 64 is (g, t):
        # partition p in block -> actual segment rank*64 + (p % 8) * 8 + p // 8
        pp = np.arange(64)
        perm64 = (pp % 8) * 8 + pp // 8
        seg = (np.arange(256).reshape(4, 64)[:, perm64]).reshape(2, 128)
        gm = (qi[None, :, None, :] >= (16 * seg[:, None, :, None] + 15))
        # gm dims [bb, Q, seg, tok] -> [seg, Q, bb, tok]
        gmask = np.ascontiguousarray(
            gm.transpose(2, 1, 0, 3)).astype(np.float32)
        im = dict(xt=_bf16(xtc), wq=wq_b, wkv=wkv_b, wo=wo_b, ident=ident_b,
                  seg16=seg16_b, tri8=tri8_b, halom=_bf16(halom),
                  gmask=_bf16(gmask), stats_lhsT=stats_b,
                  stats2_lhsT=stats2_b, ones1=ones1_b)
        if nonzero_bq:
            im["bqs"] = np.ascontiguousarray(
                (bq * SCALE).reshape(8, P).T.astype(np.float32))
        if nonzero_bkv:
            im["bkvs"] = np.ascontiguousarray(
                bkv.reshape(8, P).T.astype(np.float32))
        if nontrivial_ln_l:
            im["lnl_row"] = _bf16(np.broadcast_to(
                np.stack([ln_l_g, ln_l_b]), (P, 2, D)))
        if nontrivial_ln_g:
            im["lng_row"] = np.ascontiguousarray(np.broadcast_to(
                np.stack([ln_g_g, ln_g_b]), (64, 2, D)).astype(np.float32))
        in_maps.append(im)

    res = run_bass_kernel_spmd(nc, in_maps, list(range(NC)))
    out = np.empty((B, N, DIM), np.float32)
    for c in range(NC):
        bc, ci = c // 4, c % 4
        out[bc, ci * TOK:(ci + 1) * TOK] = res.results[c]["out"].reshape(
            TOK, DIM)
    if np.any(bo != 0.0):
        out += bo
    return out
